# revision 1
# baseline (speedup 1.0000x reference)
"""Trainium2 Bass kernel for nn_Classifier0 (quadrant-sum classifier).

reference:
    agg[n, q]  = quadrant sums of x[n] (512x512, quadrants of 256x256)
    w          = g * v[..., 0] / ||v||            [4, 4]
    y          = agg[:, :, None] * w + b_fgl      [N, 4, 4]
    out        = y.reshape(N, 16) @ W_fc.T + b_fc [N, 10]

Algebraic refactor (exact in real arithmetic):
    out[n, c] = sum_q agg[n, q] * A[q, c] + cc[c]
      A[q, c] = sum_j w[q, j] * W_fc[c, 4q + j]         (4 x 10, host, fp64)
      cc[c]   = b_fgl.ravel() @ W_fc[c] + b_fc[c]       (10, host, fp64)

Device work (data-parallel, 32 samples per core, C=2 samples per chunk):
  - per chunk: one contiguous 2 MB DMA into a [128, 4096] tile
    (partition p holds 8 consecutive image rows of sample p // 64;
    p % 64 < 32 is the image's top half).
  - DVE tensor_reduce sums the left 256 columns of each row, ACT
    (in-place activation Copy with accum_out) sums the right 256
    -> bufL/bufR [128, 16].
  - quadrant contraction + the tiny fc = PSUM accumulation of 3 matmuls
    with zero-masked weights (the mask isolates the two samples
    interleaved in the partition dim); psum [16, 20] row-major equals
    y [32, 10] with n = 2k + j.  Chunks 0..13 are finished (matmul +
    copy + DMA) early, hidden under the tail of the x stream; only
    chunks 14..15 sit on the critical tail.

Per-core stream is SBUF-fabric bound: 16 SDMA engines x ~26.5 GB/s
= ~424 GB/s -> 33.6 MB in ~79 us; with ~7 us startup preamble and
~9 us tail (last reduce, fc, output DMA receipt, engine drains) a
clean core lands at ~96 us.  On some executions one SDMA engine of a
core is slowed ~20% by roaming system traffic (~+14 us); descriptors
are round-robined over engines by the outer AP dim, so this can be
countered neither by layout nor (since it moves between runs) by
uneven sharding.
"""

import numpy as np

N, S = 256, 512
H = S // 2
NCORES = 8
SPC = N // NCORES  # samples per core (32)
NCLS = 10

C = 2  # samples per DMA chunk (bulk)
NCH2 = 14  # C=2 chunks per core (samples 0..27)
NT = 4  # single-sample tail chunks (samples 28..31)
PPS = 128 // C  # partitions per sample in a C=2 chunk (64)
RPP = S // PPS  # image rows per partition (8)
FREE = S * RPP  # floats per partition per C=2 chunk (4096)
FREE1 = S * 4  # floats per partition per C=1 chunk (2048)

_PROGRAM_CACHE = {}


def _build_program():
    from contextlib import ExitStack

    import concourse.bacc as bacc
    import concourse.mybir as mybir
    import concourse.tile as tile

    nc = bacc.Bacc("TRN2", target_bir_lowering=False, debug=False)
    dt = mybir.dt.float32

    x_t = nc.dram_tensor("x", [NCH2, 128, FREE], dt, kind="ExternalInput")
    x1_t = nc.dram_tensor("x1", [NT, 128, FREE1], dt, kind="ExternalInput")
    # all folded params packed into one tensor: cols 0:20 walm, 20:40 warm,
    # 40:50 walm1, 50:60 warm1; row 0 cols 60:80 ccbt, 80:90 ccbt1
    cst_t = nc.dram_tensor("cst", [128, 90], dt, kind="ExternalInput")
    y_t = nc.dram_tensor("y", [SPC, NCLS], dt, kind="ExternalOutput")

    with tile.TileContext(nc) as tc, ExitStack() as ctx:
        xpool = ctx.enter_context(tc.tile_pool(name="xp", bufs=8))
        cpool = ctx.enter_context(tc.tile_pool(name="cp", bufs=1))
        ppool = ctx.enter_context(tc.tile_pool(name="pp", bufs=1, space="PSUM"))

        x_ap = x_t.ap()
        x1_ap = x1_t.ap()
        # first 28 y rows viewed as [14 chunks, 20]
        y2 = y_t.ap()[0 : C * NCH2, :].rearrange("(k j) c -> k (j c)", j=C)

        bufL = cpool.tile([128, NCH2], dt)
        bufR = cpool.tile([128, NCH2], dt)
        bufL1 = cpool.tile([128, NT], dt)
        bufR1 = cpool.tile([128, NT], dt)
        # one constant load on the scalar engine's HWDGE ring: the SP ring
        # starts streaming x immediately and GpSimd stays fully idle
        cst = cpool.tile([128, 90], dt)
        nc.scalar.dma_start(cst[:], cst_t.ap())
        walm, warm = cst[:, 0:20], cst[:, 20:40]
        walm1, warm1 = cst[:, 40:50], cst[:, 50:60]
        ccbt, ccbt1 = cst[0:1, 60:80], cst[0:1, 80:90]
        ones1 = cpool.tile([1, NCH2], dt)
        nc.vector.memset(ones1[:], 1.0)

        for k in range(NCH2):
            xt = xpool.tile([128, FREE], dt)
            nc.sync.dma_start(xt[:], x_ap[k])
            xv = xt[:].rearrange("p (r c) -> p r c", c=S)
            nc.vector.tensor_reduce(
                bufL[:, k : k + 1],
                xv[:, :, 0:H],
                axis=mybir.AxisListType.XY,
                op=mybir.AluOpType.add,
            )
            nc.scalar.activation(
                xv[:, :, H:S],
                xv[:, :, H:S],
                mybir.ActivationFunctionType.Copy,
                accum_out=bufR[:, k : k + 1],
            )

        # single-sample tail chunks: half-size reduces on the critical tail
        for k in range(NT):
            xt1 = xpool.tile([128, FREE1], dt, tag="x1t")
            nc.sync.dma_start(xt1[:], x1_ap[k])
            xv1 = xt1[:].rearrange("p (r c) -> p r c", c=S)
            nc.vector.tensor_reduce(
                bufL1[:, k : k + 1],
                xv1[:, :, 0:H],
                axis=mybir.AxisListType.XY,
                op=mybir.AluOpType.add,
            )
            nc.scalar.activation(
                xv1[:, :, H:S],
                xv1[:, :, H:S],
                mybir.ActivationFunctionType.Copy,
                accum_out=bufR1[:, k : k + 1],
            )

        # C=2 chunks: all ready before the x stream drains — hidden
        psumA = ppool.tile([NCH2, C * NCLS], dt)
        nc.tensor.matmul(psumA[:], lhsT=bufL[:], rhs=walm, start=True, stop=False)
        nc.tensor.matmul(psumA[:], lhsT=bufR[:], rhs=warm, start=False, stop=False)
        nc.tensor.matmul(psumA[:], lhsT=ones1[:], rhs=ccbt, start=False, stop=True)
        outA = cpool.tile([NCH2, C * NCLS], dt)
        nc.vector.tensor_copy(outA[:], psumA[:])
        nc.sync.dma_start(y2[:], outA[:])

        # tail samples 28..31: the short critical tail
        ones2 = ones1[:, 0:NT]
        psumB = ppool.tile([NT, NCLS], dt)
        nc.tensor.matmul(psumB[:], lhsT=bufL1[:], rhs=walm1, start=True, stop=False)
        nc.tensor.matmul(psumB[:], lhsT=bufR1[:], rhs=warm1, start=False, stop=False)
        nc.tensor.matmul(psumB[:], lhsT=ones2, rhs=ccbt1, start=False, stop=True)
        outB = cpool.tile([NT, NCLS], dt)
        nc.vector.tensor_copy(outB[:], psumB[:])
        nc.sync.dma_start(y_t.ap()[C * NCH2 : SPC, :], outB[:])

    nc.compile()
    return nc


def _host_params(v, g, b_fgl, W_fc, b_fc):
    """Fold the tiny params into zero-masked walm/warm [128, C*10], cc [1, C*10]."""
    v64 = v.astype(np.float64)
    w = g.astype(np.float64) * (v64[..., 0] / np.linalg.norm(v64, axis=-1))  # [4,4]
    A = np.einsum("qj,cqj->qc", w, W_fc.astype(np.float64).reshape(NCLS, 4, 4))
    cc = b_fgl.astype(np.float64).reshape(-1) @ W_fc.astype(np.float64).T
    cc = cc + b_fc.astype(np.float64)

    # quadrant ids: 0=TL, 1=BL, 2=BR, 3=TR
    def masks(pps, c):
        p = np.arange(128)
        top = (p % pps) < (pps // 2)
        al_col = np.where(top[:, None], A[0][None, :], A[1][None, :])
        ar_col = np.where(top[:, None], A[3][None, :], A[2][None, :])
        grp = p // pps
        wl = np.zeros((128, c * NCLS))
        wr = np.zeros((128, c * NCLS))
        for j in range(c):
            sel = grp == j
            wl[sel, j * NCLS : (j + 1) * NCLS] = al_col[sel]
            wr[sel, j * NCLS : (j + 1) * NCLS] = ar_col[sel]
        cb = np.tile(cc, c).reshape(1, c * NCLS)
        return (
            np.ascontiguousarray(wl, dtype=np.float32),
            np.ascontiguousarray(wr, dtype=np.float32),
            np.ascontiguousarray(cb, dtype=np.float32),
        )

    return masks(PPS, C), masks(128, 1)


def _run(inputs, trace=False):
    from concourse.bass_utils import run_bass_kernel_spmd

    if "nc" not in _PROGRAM_CACHE:
        _PROGRAM_CACHE["nc"] = _build_program()
    nc = _PROGRAM_CACHE["nc"]

    x = np.ascontiguousarray(np.asarray(inputs["x"], dtype=np.float32))
    (walm, warm, ccbt), (walm1, warm1, ccbt1) = _host_params(
        np.asarray(inputs["v"], np.float32),
        np.asarray(inputs["g"], np.float32),
        np.asarray(inputs["b_fgl"], np.float32),
        np.asarray(inputs["W_fc"], np.float32),
        np.asarray(inputs["b_fc"], np.float32),
    )

    cst = np.zeros((128, 90), np.float32)
    cst[:, 0:20] = walm
    cst[:, 20:40] = warm
    cst[:, 40:50] = walm1
    cst[:, 50:60] = warm1
    cst[0, 60:80] = ccbt[0]
    cst[0, 80:90] = ccbt1[0]
    x_sh = x.reshape(NCORES, SPC * S * S)
    nb = C * NCH2 * S * S  # floats in the C=2 part
    in_maps = [
        {
            "x": x_sh[i, :nb].reshape(NCH2, 128, FREE),
            "x1": x_sh[i, nb:].reshape(NT, 128, FREE1),
            "cst": cst,
        }
        for i in range(NCORES)
    ]
    res = run_bass_kernel_spmd(nc, in_maps, list(range(NCORES)), trace=trace)
    y = np.concatenate([res.results[i]["y"] for i in range(NCORES)], axis=0)
    return y, res.exec_time_ns


def kernel(**inputs) -> np.ndarray:
    y, _ = _run(inputs, trace=False)
    return y

